# revision 15
# baseline (speedup 1.0000x reference)
"""Trainium2 Bass kernel for nn_ADModule_43671227466565 (vq_codebook).

Data-parallel over the N axis on 8 NeuronCores. Each core handles 8192 rows:
  logits = x @ w.T          (PE, float32r full-rate fp32)
  att    = l1norm(hardshrink(softmax(logits)))   (ScalarE exp + DVE mask)
  out    = att @ w          (PE, via on-chip transpose of att)
  nl/nl2 = w[argmax/arg2nd(att)]  (DVE max8/max_index + DMA row gather)
  col    = ||x - mean(w)|| < 1    (GpSimd)

Self-contained: hardcodes all shapes; only needs numpy + the concourse stack.
"""
import sys
import numpy as np

sys.path.insert(0, "/opt/trn_rl_repo")

import concourse.bass as bass
import concourse.bacc as bacc
import concourse.tile as tile
import concourse.mybir as mybir
from concourse.bass_utils import run_bass_kernel_spmd
from contextlib import ExitStack

dt = mybir.dt
Alu = mybir.AluOpType
Act = mybir.ActivationFunctionType

N, D, L = 65536, 128, 2048
NCORES = 8
SH = N // NCORES          # 8192 rows per core
THRES = 0.0025
EPS = 1e-12

P = 128                   # partitions / tile rows
LCH = L // P              # 16 l-chunks
MMB = 4                   # tiles per MM2 batch (moving free = 512)
GB = 8                    # tiles per nl-gather batch


def f32r(ap):
    return ap.bitcast(dt.float32r)


def build_kernel(nt: int):
    """nt = number of 128-row tiles per core (64 full scale)."""
    sh = nt * P
    nc = bacc.Bacc("TRN2", target_bir_lowering=False, debug=False)

    x_ext = nc.declare_dram_parameter("input", [sh, D], dt.float32, isOutput=False)
    w_ext = nc.declare_dram_parameter("weight", [L, D], dt.float32, isOutput=False)
    out_ext = nc.declare_dram_parameter("out", [sh, D], dt.float32, isOutput=True)
    att_ext = nc.declare_dram_parameter("att", [sh, L], dt.float32, isOutput=True)
    nl_ext = nc.declare_dram_parameter("nl", [sh, D], dt.float32, isOutput=True)
    nl2_ext = nc.declare_dram_parameter("nl2", [sh, D], dt.float32, isOutput=True)
    col_ext = nc.declare_dram_parameter("col", [sh], dt.uint8, isOutput=True)
    id_ext = nc.declare_dram_parameter("ident128", [P, P], dt.float32,
                                       isOutput=False)

    gb = min(GB, nt)
    mmb = min(MMB, nt)

    with tile.TileContext(nc) as tc, ExitStack() as ctx:
        # ---- pools ----
        const_pool = ctx.enter_context(tc.tile_pool(name="const", bufs=1))
        xp = ctx.enter_context(tc.tile_pool(name="x", bufs=3))
        ep = ctx.enter_context(tc.tile_pool(name="e", bufs=2))
        mep = ctx.enter_context(tc.tile_pool(name="me", bufs=2))
        attp = ctx.enter_context(tc.tile_pool(name="att", bufs=4))
        attTp = ctx.enter_context(tc.tile_pool(name="attT", bufs=2))
        outp = ctx.enter_context(tc.tile_pool(name="outsb", bufs=2))
        nlp = ctx.enter_context(tc.tile_pool(name="nlg", bufs=2))
        idxp = ctx.enter_context(tc.tile_pool(name="idx", bufs=2))
        smp = ctx.enter_context(tc.tile_pool(name="small", bufs=4))
        ps_log = ctx.enter_context(tc.tile_pool(name="pslog", bufs=1, space="PSUM"))
        ps_attT = ctx.enter_context(tc.tile_pool(name="psattT", bufs=2, space="PSUM"))
        ps_out = ctx.enter_context(tc.tile_pool(name="psout", bufs=1, space="PSUM"))
        ps_sm = ctx.enter_context(tc.tile_pool(name="pssm", bufs=1, space="PSUM"))
        dramp = ctx.enter_context(tc.tile_pool(name="dram", bufs=2, space="DRAM"))

        # ---- constants / setup ----
        ident_f = const_pool.tile([P, P], dt.float32)
        nc.sync.dma_start(ident_f[:], id_ext[:])
        ident = const_pool.tile([P, P], dt.float32r)
        nc.scalar.copy(ident[:], ident_f[:])
        ones_col_f = const_pool.tile([P, 1], dt.float32)
        nc.gpsimd.memset(ones_col_f[:], 1.0)
        ones_col = const_pool.tile([P, 1], dt.float32r)
        nc.scalar.copy(ones_col[:], ones_col_f[:])
        ones_row_f = const_pool.tile([1, P], dt.float32)
        nc.gpsimd.memset(ones_row_f[:], 1.0)
        ones_row = const_pool.tile([1, P], dt.float32r)
        nc.scalar.copy(ones_row[:], ones_row_f[:])

        # codebook resident in SBUF: w_sb[p, c, d] = w[c*128+p, d]
        w_f = const_pool.tile([P, LCH, D], dt.float32)
        nc.sync.dma_start(w_f[:], w_ext.rearrange("(c p) d -> p c d", p=P))
        w_sb = const_pool.tile([P, LCH, D], dt.float32r)
        nc.scalar.copy(w_sb[:], w_f[:])
        # wT[d, l] via PE transposes (fp32 exact, feeds fp32 MM1)
        wT = const_pool.tile([P, L], dt.float32)
        for g in range(LCH // 4):
            pst_f = ps_attT.tile([P, 512], dt.float32, tag="psattT")
            for c in range(4):
                k = g * 4 + c
                nc.tensor.transpose(
                    pst_f[:, c * P:(c + 1) * P], w_f[:, k, :], ident_f[:])
            nc.scalar.copy(wT[:, g * 512:(g + 1) * 512], pst_f[:])

        # cen = mean(w) broadcast to [128, 128]
        ps_cen = ps_sm.tile([1, P], dt.float32, tag="pssm")
        for k in range(LCH):
            nc.tensor.matmul(ps_cen[:], ones_col[:], w_sb[:, k, :],
                             start=(k == 0), stop=(k == LCH - 1))
        cen_row = const_pool.tile([1, P], dt.float32r)
        nc.scalar.mul(cen_row[:], ps_cen[:], 1.0 / L)
        ps_cb = ps_sm.tile([P, P], dt.float32, tag="pssm")
        nc.tensor.matmul(ps_cb[:], ones_row[:], cen_row[:])
        cen_b = const_pool.tile([P, P], dt.float32)
        nc.scalar.copy(cen_b[:], ps_cb[:])

        col_acc = const_pool.tile([P, nt], dt.float32)
        gsem = nc.alloc_semaphore("gather_sem")
        gcnt = [0]

        i1acc = i2acc = None
        attT = None

        for t in range(nt):
            tb = t % mmb            # position in MM2 batch
            tg = t % gb             # position in gather batch
            if tg == 0:
                i1acc = idxp.tile([P, gb], dt.int16, tag="i1acc")
                i2acc = idxp.tile([P, gb], dt.int16, tag="i2acc")
            if tb == 0:
                attT = attTp.tile([P, LCH, mmb, P], dt.float32r, tag="attT")

            # -- load x tile, transpose on PE --
            x_sb = xp.tile([P, D], dt.float32, tag="x")
            nc.sync.dma_start(x_sb[:], x_ext[bass.ts(t, P), :])
            ps_xt = ps_sm.tile([P, P], dt.float32, tag="pssm")
            nc.tensor.transpose(ps_xt[:], x_sb[:], ident_f[:])
            xT = xp.tile([P, P], dt.float32, tag="xT")
            nc.scalar.copy(xT[:], ps_xt[:])

            # -- MM1: logits[n, l] (fp32r full rate) --
            ps_l = ps_log.tile([P, L], dt.float32, tag="pslog")
            for j in range(4):
                nc.tensor.matmul(ps_l[:, j * 512:(j + 1) * 512], xT[:],
                                 wT[:, j * 512:(j + 1) * 512])

            # -- exp + row-sum S (ScalarE, fused accumulate) --
            e = ep.tile([P, L], dt.float32, tag="e")
            S = smp.tile([P, 1], dt.float32, tag="S")
            nc.scalar.activation(e[:], ps_l[:], Act.Exp, accum_out=S[:])

            # -- mask: me = (e > t*S) * e ; G = sum(me) --
            tS = smp.tile([P, 1], dt.float32, tag="tS")
            nc.vector.tensor_scalar_mul(tS[:], S[:], THRES)
            epsS = smp.tile([P, 1], dt.float32, tag="epsS")
            nc.vector.tensor_scalar_mul(epsS[:], S[:], EPS)
            me = mep.tile([P, L], dt.float32, tag="me")
            G = smp.tile([P, 1], dt.float32, tag="G")
            nc.vector.scalar_tensor_tensor(
                me[:], e[:], tS[:], e[:], op0=Alu.is_gt, op1=Alu.mult,
                accum_out=G[:])

            # -- r2 = 1 / max(G, eps*S) ; att = me * r2 --
            Gm = smp.tile([P, 1], dt.float32, tag="Gm")
            nc.vector.tensor_tensor(Gm[:], G[:], epsS[:], op=Alu.max)
            r2 = smp.tile([P, 1], dt.float32, tag="r2")
            nc.vector.reciprocal(r2[:], Gm[:])
            att = attp.tile([P, L], dt.float32r, tag="att")
            nc.vector.tensor_scalar(att[:], me[:], r2[:], None, op0=Alu.mult)
            for j in range(4):
                nc.sync.dma_start(att_ext[bass.ts(t, P), bass.ts(j, 512)],
                                  att[:, bass.ts(j, 512)].bitcast(dt.float32))

            # -- top-8 values + indices (argmax / 2nd argmax) --
            t8 = smp.tile([P, 8], dt.float32, tag="t8")
            nc.vector.max(t8[:], me[:])
            i8 = smp.tile([P, 8], dt.uint32, tag="i8")
            nc.vector.max_index(i8[:], t8[:], me[:])
            # ind = i8[:,0] (covers empty rows: first zero is index 0)
            nc.vector.tensor_copy(i1acc[:, tg:tg + 1], i8[:, 0:1])
            # ind2 = i8[:,1] if t8[:,1] > 0 else 0
            f2 = smp.tile([P, 1], dt.float32, tag="f2")
            nc.vector.tensor_scalar(f2[:], t8[:, 1:2], 0.0, None, op0=Alu.is_gt)
            i8f = smp.tile([P, 1], dt.float32, tag="i8f")
            nc.vector.tensor_copy(i8f[:], i8[:, 1:2])
            ind2f = smp.tile([P, 1], dt.float32, tag="ind2f")
            nc.vector.tensor_tensor(ind2f[:], i8f[:], f2[:], op=Alu.mult)
            nc.vector.tensor_copy(i2acc[:, tg:tg + 1], ind2f[:])

            # -- col: ||x - cen||^2 < 1 (GpSimd) --
            diff = xp.tile([P, D], dt.float32, tag="diff")
            nc.gpsimd.tensor_tensor(diff[:], x_sb[:], cen_b[:], op=Alu.subtract)
            junk = xp.tile([P, D], dt.float32, tag="junk")
            ss = smp.tile([P, 1], dt.float32, tag="ss")
            nc.vector.scalar_tensor_tensor(
                junk[:], diff[:], 1.0, diff[:], op0=Alu.mult, op1=Alu.mult,
                accum_out=ss[:])
            nc.vector.tensor_scalar(col_acc[:, t:t + 1], ss[:], 1.0, None,
                                    op0=Alu.is_lt)

            # -- transpose att into [l, n] chunks for MM2 --
            for g in range(LCH // 4):
                pst = ps_attT.tile([P, 512], dt.float32r, tag="psattT")
                for c in range(4):
                    k = g * 4 + c
                    nc.tensor.transpose(
                        pst[:, c * P:(c + 1) * P],
                        att[:, k * P:(k + 1) * P], ident[:])
                # scatter the 4 chunks into attT[:, 4g:4g+4, tb, :]
                nc.scalar.copy(attT[:, g * 4:(g + 1) * 4, tb, :], pst[:])

            # -- MM2 (once per batch): outT[d, n_batch] = sum_l w[l,d] attT[l,n] --
            if tb == mmb - 1:
                ps_o = ps_out.tile([P, P * mmb], dt.float32, tag="psout")
                for k in range(LCH):
                    nc.tensor.matmul(ps_o[:], w_sb[:, k, :],
                                     attT[:, k, :, :],
                                     start=(k == 0), stop=(k == LCH - 1))
                oT_sb = outp.tile([P, P * mmb], dt.float32r, tag="oT")
                nc.scalar.copy(oT_sb[:], ps_o[:])
                # transpose back to [n, d] and DMA out
                for j in range(mmb):
                    ps_o2 = ps_sm.tile([P, P], dt.float32r, tag="pssm")
                    nc.tensor.transpose(ps_o2[:], oT_sb[:, bass.ts(j, P)],
                                        ident[:])
                    o_sb = outp.tile([P, P], dt.float32, tag="o2")
                    nc.scalar.copy(o_sb[:], ps_o2[:])
                    nc.sync.dma_start(out_ext[bass.ts(t - mmb + 1 + j, P), :],
                                      o_sb[:])

            # -- nl/nl2 row gathers (once per gather batch) --
            if tg == gb - 1:
                b0 = (t - gb + 1) * P     # first row of this batch
                for (acc, dst_ext, qn) in ((i1acc, nl_ext, 0), (i2acc, nl2_ext, 0)):
                    bounce = dramp.tile([P * gb], dt.int16, tag="bounce")
                    # bounce[c*128 + p] = acc[p, c]
                    nc.sync.dma_start(
                        bounce.rearrange("(c p) -> p c", p=P), acc[:])
                    idxs = idxp.tile([P, P * gb // 16], dt.int16, tag="idxs")
                    src = bounce.rearrange("(c r) -> r c", r=16)
                    for grp in range(8):
                        nc.sync.dma_start(idxs[grp * 16:(grp + 1) * 16, :], src)
                    nlg = nlp.tile([P, gb, D], dt.float32, tag="nlg")
                    with tc.tile_critical():
                        gcnt[0] += 16
                        nc.gpsimd.dma_gather(
                            nlg[:], w_ext[:], idxs[:], num_idxs=P * gb,
                            num_idxs_reg=P * gb, elem_size=D,
                            queue_num=qn).then_inc(gsem, 16)
                        nc.gpsimd.wait_ge(gsem, gcnt[0])
                    nc.sync.dma_start(
                        dst_ext[b0:b0 + P * gb, :].rearrange(
                            "(j p) d -> p j d", p=P),
                        nlg[:])

        # ---- col epilogue: transpose to [t, p] and emit u8 ----
        col_r = const_pool.tile([P, nt], dt.float32r)
        nc.scalar.copy(col_r[:], col_acc[:])
        ps_ct = ps_sm.tile([P, P], dt.float32r, tag="pssm")
        nc.tensor.transpose(ps_ct[0:nt, :], col_r[:], ident[:])
        colT = const_pool.tile([nt, P], dt.uint8)
        nc.scalar.copy(colT[:], ps_ct[0:nt, :].bitcast(dt.float32))
        nc.sync.dma_start(col_ext.rearrange("(t p) -> t p", p=P), colT[:])

    nc.finalize()
    return nc


_NC_CACHE = {}


def _install_ntff_hook():
    """Provide antenv.axon_hooks with a ctypes NTFF profiling hook so
    run_bass_kernel_spmd(trace=True) can return exec_time_ns under axon."""
    import types, ctypes, contextlib, os
    try:
        from antenv.axon_hooks import get_axon_ntff_profile_hook  # noqa
        return  # already present
    except ImportError:
        pass
    so_path = "/opt/axon/libaxon_pjrt.so"
    if not os.path.exists(so_path):
        return
    lib = ctypes.CDLL(so_path)
    if not hasattr(lib, "axon_start_nrt_profile"):
        return
    lib.axon_start_nrt_profile.argtypes = [
        ctypes.POINTER(ctypes.c_int64), ctypes.c_size_t]
    lib.axon_start_nrt_profile.restype = ctypes.c_int64
    lib.axon_stop_nrt_profile.argtypes = [ctypes.c_char_p]
    lib.axon_stop_nrt_profile.restype = ctypes.c_int64

    @contextlib.contextmanager
    def _hook(output_dir, device_ids):
        import jax
        jax.devices()
        if device_ids:
            ids = (ctypes.c_int64 * len(device_ids))(*device_ids)
            rc = lib.axon_start_nrt_profile(ids, len(device_ids))
        else:
            rc = lib.axon_start_nrt_profile(None, 0)
        if rc != 0:
            raise RuntimeError(f"axon_start_nrt_profile rc={rc}")
        try:
            yield
        finally:
            n = lib.axon_stop_nrt_profile(str(output_dir).encode())
            print(f"ntff profile: {n} file(s) -> {output_dir}")

    import antenv
    mod = types.ModuleType("antenv.axon_hooks")
    _h = [_hook]
    mod.set_axon_ntff_profile_hook = lambda h: _h.__setitem__(0, h)
    mod.get_axon_ntff_profile_hook = lambda: _h[0]
    import sys as _s
    _s.modules["antenv.axon_hooks"] = mod
    antenv.axon_hooks = mod


def kernel(input: np.ndarray, weight: np.ndarray):
    x = np.ascontiguousarray(input, dtype=np.float32)
    w = np.ascontiguousarray(weight, dtype=np.float32)
    assert x.shape == (N, D) and w.shape == (L, D)

    if "nc" not in _NC_CACHE:
        _NC_CACHE["nc"] = build_kernel(N // NCORES // P)
    nc = _NC_CACHE["nc"]

    ident = np.eye(P, dtype=np.float32)
    in_maps = [
        {"input": x[i * SH:(i + 1) * SH], "weight": w, "ident128": ident}
        for i in range(NCORES)
    ]
    import os
    trace = bool(os.environ.get("VQ_TRACE"))
    if trace:
        try:
            _install_ntff_hook()
        except Exception as ex:
            print("ntff hook install failed:", ex)
            trace = False
    res = run_bass_kernel_spmd(nc, in_maps, core_ids=list(range(NCORES)),
                               trace=trace)
    _NC_CACHE["exec_time_ns"] = res.exec_time_ns
    _NC_CACHE["profile"] = res.profile_json
    results = res.results

    output = np.concatenate([results[i]["out"] for i in range(NCORES)], axis=0)
    att = np.concatenate([results[i]["att"] for i in range(NCORES)], axis=0)
    nl = np.concatenate([results[i]["nl"] for i in range(NCORES)], axis=0)
    nl2 = np.concatenate([results[i]["nl2"] for i in range(NCORES)], axis=0)
    col = np.concatenate(
        [results[i]["col"] for i in range(NCORES)], axis=0).astype(bool)
    return output, att, nl, nl2, col
